# revision 1
# baseline (speedup 1.0000x reference)
"""BorderLoss Trainium2 kernel.

Reference (per element, then global mean over [64,512,512]):
    loss = softplus(x) - x*y          (y binary {0,1})
    m = (y > 0);  ero = 3x3 min-pool(m);  dil = 3x3 max-pool(m)  (SAME, OOB
    ignored);  w = 1 + (dil - ero);  out = mean(loss * w)

Key identities used:
  * loss = softplus((1-2y)*x)  (stable BCE identity) -> loss is a pure
    activation chain Ln(Exp(z)+1) on the Scalar engine, z = (1-2y)*x.
  * With s = 3x3 box-count of ones (OOB=0) and cnt = #in-bounds cells,
    border = dil-ero = [1 <= s <= cnt-1]. For a row with rv in-bounds
    window rows and interior columns, cnt = 3*rv and
    border <=> |s - mu|/rho <= 1 with mu = 1.5*rv, rho = 1.5*rv - 0.75.
    The tridiagonal vertical-sum matmul is pre-scaled per output row by
    1/rho and shifted by -mu/rho (rank-1 ones term), so the on-chip
    border test is a single |s''| <= 1 tensor-scalar op. Edge columns
    (cv=2) only over-count when s == 2*rv, fixed by one tiny fused op on
    columns {0,511} accumulating sum(l * [s'' >= 0.26]).

Per core (8 images, data parallel across 8 NeuronCores):
  - m = cast-DMA of y (int32->bf16), z = ts(m*-2+1) then cast-DMA of x
    with accum_op=mult (z = (1-2y)*x, no DVE pass for the product).
  - hs = horizontal 3-tap sum: gpsimd add (right neighbor) + SBUF->SBUF
    DMA accumulate (left neighbor).
  - s'' = scaled/shifted vertical 3-tap via PE matmuls into PSUM
    (tridiag variants + U/L cross-block single-entry mats + rank-1 -mu/rho).
  - l = Ln(Exp(z)+1) on ACT with accum_out giving sum(l) free.
  - border = ts(|s''| <= 1) [DVE], lb = l*border [DVE 2x TT],
    sum(lb) via ones-matmul on PE into a persistent PSUM bank.
  - edge fix: one tiny STT per half-image.
Host combines: total = sum(l) + sum(lb) - sum(edge);  mean = total/N/H/W.
"""

import sys
import numpy as np

if "/opt/trn_rl_repo" not in sys.path:
    sys.path.insert(0, "/opt/trn_rl_repo")

H = W = 512
P = 128
NB = 4              # row blocks per image
DBLK = 512
FI = NB * DBLK      # 2048 free cols per image (dense)
NACC = 7            # per img: sum(l), dil h0/h1, ero h0/h1, edge h0/h1
N_CORES = 8
EDGE_THR = 0.26

_CACHE = {}


def _consts():
    import ml_dtypes
    bf = ml_dtypes.bfloat16
    # per-block-type row params: rv (in-bounds window rows) per partition
    rv = np.full((NB, P), 3.0, dtype=np.float64)
    rv[0, 0] = 2.0
    rv[NB - 1, P - 1] = 2.0
    mu = 1.5 * rv                       # [NB, P]
    rho = 1.5 * rv - 0.75
    w = (1.0 / rho)                     # per output-row scale

    tri0 = np.zeros((P, P), dtype=np.float64)
    for k in range(P):
        tri0[k, max(0, k - 1):min(P, k + 2)] = 1.0
    u0 = np.zeros((P, P), dtype=np.float64)
    u0[0, P - 1] = 1.0                  # next block's row 0 -> out row 127
    l0 = np.zeros((P, P), dtype=np.float64)
    l0[P - 1, 0] = 1.0                  # prev block's row 127 -> out row 0

    # tri const [P, 5*P]: T0 scaled for blk0 / mid / blk3, then U, L
    tri = np.zeros((P, 5 * P), dtype=bf)
    for t, b in enumerate((0, 1, NB - 1)):
        tri[:, t * P:(t + 1) * P] = (tri0 * w[b][None, :]).astype(bf)
    tri[:, 3 * P:4 * P] = (u0 * w[1][None, :]).astype(bf)   # target rows rv=3
    tri[:, 4 * P:5 * P] = (l0 * w[1][None, :]).astype(bf)

    # aux const [P, 5*P]: row 0 cols [b*P:(b+1)*P] = -mu/rho for block b;
    # col 4*P.. : ones row [1, DBLK] at row 0; col 0 of cols... use layout:
    #   aux[0, b*P + m] = -mu/rho (blocks 0..3)
    #   aux[:, 4*P:4*P+1] = 1.0 (ones column, lhsT for lb reduction)
    #   aux[0, 4*P+1 : 4*P+1+DBLK] would exceed; use separate region below.
    aux = np.zeros((P, 5 * P + DBLK), dtype=bf)
    for b in range(NB):
        aux[0, b * P:(b + 1) * P] = (-mu[b] / rho[b]).astype(bf)
    aux[:, 4 * P] = bf(1.0)                      # ones column [P,1]
    aux[0, 4 * P + 1:4 * P + 1 + DBLK] = bf(1.0)  # ones row [1, DBLK]
    return tri, aux


def _build(n_imgs):
    import concourse.bass as bass
    import concourse.bacc as bacc
    import concourse.tile as tile
    from concourse import mybir

    f32 = mybir.dt.float32
    bf16 = mybir.dt.bfloat16
    i32 = mybir.dt.int32
    Alu = mybir.AluOpType
    Act = mybir.ActivationFunctionType

    nc = bacc.Bacc(None, target_bir_lowering=False)
    x_d = nc.dram_tensor("x", [n_imgs, H, W], f32, kind="ExternalInput")
    y_d = nc.dram_tensor("y", [n_imgs, H, W], i32, kind="ExternalInput")
    tri_d = nc.dram_tensor("tri", [P, 5 * P], bf16, kind="ExternalInput")
    aux_d = nc.dram_tensor("aux", [P, 5 * P + DBLK], bf16, kind="ExternalInput")
    acc_d = nc.dram_tensor("acc", [P, n_imgs * NACC], f32, kind="ExternalOutput")

    with tile.TileContext(nc) as tc:
        with (
            tc.tile_pool(name="consts", bufs=1) as cpool,
            tc.tile_pool(name="io", bufs=3) as io,
            tc.tile_pool(name="work", bufs=3) as work,
            tc.tile_pool(name="accp", bufs=1) as apool,
            tc.tile_pool(name="ps", bufs=3, space=bass.MemorySpace.PSUM) as pp,
        ):
            tri = cpool.tile([P, 5 * P], bf16)
            aux = cpool.tile([P, 5 * P + DBLK], bf16)
            nc.sync.dma_start(tri[:], tri_d[:])
            nc.sync.dma_start(aux[:], aux_d[:])
            onescol = aux[:, 4 * P:4 * P + 1]          # [P,1] lhsT
            onesrow = aux[0:1, 4 * P + 1:4 * P + 1 + DBLK]  # [1,DBLK] rhs

            accs = apool.tile([P, n_imgs * NACC], f32)

            for i in range(n_imgs):
                a0 = i * NACC
                m = io.tile([P, FI], bf16, tag="m")
                zb = io.tile([P, FI], bf16, tag="zb")
                m3 = m.rearrange("p (b c) -> p b c", c=DBLK)

                # m = cast(y); z = (1-2m)*x  (walrus rejects DMA accum mult,
                # so the product is a 2x TT)
                xb = io.tile([P, FI], bf16, tag="xb")
                nc.gpsimd.dma_start(m3, y_d[i].rearrange("(b p) w -> p b w", p=P))
                nc.gpsimd.dma_start(
                    xb.rearrange("p (b c) -> p b c", c=DBLK),
                    x_d[i].rearrange("(b p) w -> p b w", p=P))
                nc.vector.tensor_scalar(zb[:], m[:], -2.0, 1.0, Alu.mult, Alu.add)
                nc.vector.tensor_mul(zb[:], zb[:], xb[:])

                # horizontal 3-tap box sum (per-block, OOB=0)
                hs = work.tile([P, FI], bf16, tag="hs")
                hs3 = hs.rearrange("p (b c) -> p b c", c=DBLK)
                nc.gpsimd.tensor_add(hs3[:, :, 0:DBLK - 1], m3[:, :, 0:DBLK - 1],
                                     m3[:, :, 1:DBLK])
                nc.gpsimd.tensor_copy(hs3[:, :, DBLK - 1:DBLK],
                                      m3[:, :, DBLK - 1:DBLK])
                nc.gpsimd.dma_start(hs3[:, :, 1:DBLK], m3[:, :, 0:DBLK - 1],
                                    accum_op=Alu.add)

                # loss on ACT: l = Ln(Exp(z)+1), accum -> sum(l)
                eb = work.tile([P, FI], bf16, tag="eb")
                lt = work.tile([P, FI], bf16, tag="lt")
                nc.scalar.activation(eb[:], zb[:], Act.Exp)
                nc.scalar.activation(lt[:], eb[:], Act.Ln, bias=1.0,
                                     accum_out=accs[:, a0:a0 + 1])

                # vertical scaled 3-tap via PE, per half-image (2 banks)
                for h in range(2):
                    sp = pp.tile([P, 2 * DBLK], f32, tag="sp")
                    for j in range(2):
                        b = 2 * h + j
                        tcol = 0 if b == 0 else (2 if b == NB - 1 else 1)
                        o = sp[:, j * DBLK:(j + 1) * DBLK]
                        mms = [(tri[:, tcol * P:(tcol + 1) * P], hs3[:, b, :])]
                        if b > 0:
                            mms.append((tri[:, 4 * P:5 * P], hs3[:, b - 1, :]))
                        if b < NB - 1:
                            mms.append((tri[:, 3 * P:4 * P], hs3[:, b + 1, :]))
                        mms.append((aux[0:1, b * P:(b + 1) * P], onesrow))
                        for k, (ltm, r) in enumerate(mms):
                            nc.tensor.matmul(o, ltm, r, start=(k == 0),
                                             stop=(k == len(mms) - 1))

                    # border = [s'' >= -1.05] - [s'' >= 1.05]; two fused
                    # one-sided products with accumulation (STT is 1x-only,
                    # abs/band ops are ISA-illegal in tensor_scalar)
                    lh = lt[:, h * 2 * DBLK:(h + 1) * 2 * DBLK]
                    u1 = work.tile([P, 2 * DBLK], bf16, tag="u1")
                    nc.vector.scalar_tensor_tensor(
                        u1[:], sp[:], -1.05, lh[:], Alu.is_ge, Alu.mult,
                        accum_out=accs[:, a0 + 1 + h:a0 + 2 + h])
                    u2 = work.tile([P, 2 * DBLK], bf16, tag="u2")
                    nc.vector.scalar_tensor_tensor(
                        u2[:], sp[:], 1.05, lh[:], Alu.is_ge, Alu.mult,
                        accum_out=accs[:, a0 + 3 + h:a0 + 4 + h])
                    # edge-column fix: sum(l * [s'' >= EDGE_THR]) cols {0,511}
                    spe = sp.rearrange("p (b c) -> p b c", c=DBLK)[:, :, ::DBLK - 1]
                    le = lh.rearrange("p (b c) -> p b c", c=DBLK)[:, :, ::DBLK - 1]
                    et = work.tile([P, 4], bf16, tag="et")
                    nc.vector.scalar_tensor_tensor(
                        et.rearrange("p (b c) -> p b c", c=2), spe, EDGE_THR, le,
                        Alu.is_ge, Alu.mult,
                        accum_out=accs[:, a0 + 5 + h:a0 + 6 + h])

            nc.sync.dma_start(acc_d[:], accs[:])

    nc.compile()
    return nc


def _get_nc(n_imgs):
    if n_imgs not in _CACHE:
        _CACHE[n_imgs] = _build(n_imgs)
    return _CACHE[n_imgs]


def _combine(acc, n_imgs):
    # total = sum(l) + sum(l*dil) - sum(l*ero) - sum(edge fix)
    a = acc.reshape(P, n_imgs, NACC).astype(np.float64)
    return (a[:, :, 0].sum() + a[:, :, 1:3].sum() - a[:, :, 3:5].sum()
            - a[:, :, 5:7].sum())


def kernel(x, y):
    from concourse import bass_utils

    n = x.shape[0]
    per = n // N_CORES
    nc = _get_nc(per)
    tri, aux = _consts()
    x = np.ascontiguousarray(x, dtype=np.float32)
    y = np.ascontiguousarray(y, dtype=np.int32)
    in_maps = [
        {"x": x[c * per:(c + 1) * per], "y": y[c * per:(c + 1) * per],
         "tri": tri, "aux": aux}
        for c in range(N_CORES)
    ]
    res = bass_utils.run_bass_kernel_spmd(nc, in_maps, core_ids=list(range(N_CORES)))
    total = 0.0
    for r in res.results:
        total += _combine(r["acc"], per)
    return np.float32(total / (n * H * W))



# revision 4
# speedup vs baseline: 1.1921x; 1.1921x over previous
"""BorderLoss Trainium2 kernel (v4).

Reference (per element, then global mean over [64,512,512]):
    loss = softplus((1-2y)*x)   (stable BCE identity, y binary)
    m = (y > 0);  border = dilate3x3(m) - erode3x3(m)  (SAME, OOB ignored)
    w = 1 + border;  out = mean(loss * w)

Scheme (validated elementwise-exact vs reference in numpy):
  * v = 3x3 box-count of m with OOB=0, computed as horizontal 3-tap then
    vertical 3-tap.  Rows 0/511 get an extra 1.5x scale (folded into the
    tridiagonal matmul weights), after which ONE uniform band test
    |v - 4.5| <= 4.05  (i.e. 1 <= v <= 8) is exact everywhere except
    columns 0/511, fixed by a single strided STT with threshold 5.5
    (which also handles the corners exactly).
  * Horizontal 3-tap: outer pair (left+right) via one DVE bf16 2x add on
    a padded layout [P, 4, 516] (pads zero); the center tap is folded
    into the vertical matmul by running every tridiag/U/L pass twice,
    once on the outer-pair tensor and once on the center view.
  * Vertical 3-tap: per 128-row block, tridiagonal matmul on PE with
    single-entry U/L matrices carrying the cross-block rows.
  * loss: zh = (m - 0.5) * x  (DVE STT), then ACT Exp(scale=-2) and
    Ln(bias=1) = softplus((1-2m)x), with accum_out giving sum(l) free.
    A patched activation-table dict pins exp/ln/abs to the one table set
    containing all of them (baseline lost 18us to per-image reloads).
  * border-weighted sum: blocks 0-2 via ACT Abs(v-4.5) then one DVE bf16
    2x STT (<=4.05)*l; block 3 via two one-sided PSUM STTs (>=0.45 minus
    >=8.55)*l to balance ACT vs DVE load.  All reductions via accum_out
    into per-image [P,5] tiles, combined on host:
      total = sum(l) + [abs-path] + [>=0.45] - [>=8.55] - [colfix]
"""

import sys
import numpy as np

if "/opt/trn_rl_repo" not in sys.path:
    sys.path.insert(0, "/opt/trn_rl_repo")

# ---- pin exp/ln/abs/square to the single covering activation-table set ----
from concourse import hw_specs as _hw
import functools as _ft

if not getattr(_hw.get_activation_tables, "_borderloss_patched", False):
    _orig_tabs = _hw.get_activation_tables.__wrapped__

    @_ft.cache
    def _patched_tabs(module_arch):
        from concourse import mybir as _mb
        A = _mb.ActivationFunctionType
        strip = {A.Exp, A.Ln, A.Abs, A.Square}
        out = {}
        for k, v in _orig_tabs(module_arch).items():
            out[k] = v if k == "natural_log_exp_and_others" else v - strip
        return out

    _patched_tabs._borderloss_patched = True
    _hw.get_activation_tables = _patched_tabs

H = W = 512
P = 128
NB = 4               # 128-row blocks per image
FB = 516             # padded block width (data at cols 2..513, zeros at 1, 514)
FI = NB * FB         # 2064 padded free cols per image
FD = NB * W          # 2048 dense free cols per image
NACC = 5
N_CORES = 8
ABS_BLOCKS = 3       # blocks on the ACT-Abs path; the rest on one-sided STTs

_CACHE = {}


def _consts():
    import ml_dtypes
    bf = ml_dtypes.bfloat16
    tri = np.zeros((P, P), dtype=np.float64)
    for k in range(P):
        tri[k, max(0, k - 1):min(P, k + 2)] = 1.0
    t0 = tri.copy()
    t0[0:2, 0] = 1.5          # scale output row 0 (global row 0) by 1.5
    t3 = tri.copy()
    t3[126:128, 127] = 1.5    # scale output row 127 of block 3 (row 511)
    u = np.zeros((P, P), dtype=np.float64)
    u[0, 127] = 1.0           # next block's row 0 -> out row 127
    lm = np.zeros((P, P), dtype=np.float64)
    lm[127, 0] = 1.0          # prev block's row 127 -> out row 0
    wts = np.concatenate([t0, tri, t3, u, lm], axis=1).astype(bf)
    return wts


def _build(n_imgs):
    import concourse.bass as bass
    import concourse.bacc as bacc
    import concourse.tile as tile
    from concourse import mybir

    f32 = mybir.dt.float32
    bf16 = mybir.dt.bfloat16
    i32 = mybir.dt.int32
    Alu = mybir.AluOpType
    Act = mybir.ActivationFunctionType

    nc = bacc.Bacc(None, target_bir_lowering=False)
    x_d = nc.dram_tensor("x", [n_imgs, H, W], f32, kind="ExternalInput")
    y_d = nc.dram_tensor("y", [n_imgs, H, W], i32, kind="ExternalInput")
    w_d = nc.dram_tensor("wts", [P, 5 * P], bf16, kind="ExternalInput")
    acc_d = nc.dram_tensor("acc", [P, n_imgs * NACC], f32, kind="ExternalOutput")

    AB = ABS_BLOCKS
    FA = AB * W              # dense cols on the abs path
    with tile.TileContext(nc) as tc:
        with (
            tc.tile_pool(name="consts", bufs=1) as cpool,
            tc.tile_pool(name="inputs", bufs=1) as ipool,
            tc.tile_pool(name="work", bufs=2) as work,
            tc.tile_pool(name="accp", bufs=1) as apool,
            tc.tile_pool(name="ps", bufs=2, space=bass.MemorySpace.PSUM) as pp,
        ):
            wts = cpool.tile([P, 5 * P], bf16)
            nc.sync.dma_start(wts[:], w_d[:])
            bias_t = cpool.tile([P, 1], f32)
            nc.vector.memset(bias_t[:], -4.5)
            W_T0 = wts[:, 0:P]
            W_TRI = wts[:, P:2 * P]
            W_T3 = wts[:, 2 * P:3 * P]
            W_U = wts[:, 3 * P:4 * P]
            W_L = wts[:, 4 * P:5 * P]

            ms, xs, accs = [], [], []
            for i in range(n_imgs):
                m = ipool.tile([P, FI], bf16, tag=f"m{i}", name=f"m{i}")
                m3 = m.rearrange("p (b c) -> p b c", c=FB)
                # zero the pad columns (slots 1 and 514 of each block)
                nc.gpsimd.memset(m3[:, :, 1:FB - 1:FB - 3], 0)
                ms.append(m)
                xs.append(ipool.tile([P, FD], bf16, tag=f"x{i}", name=f"x{i}"))
                accs.append(apool.tile([P, NACC], f32, tag=f"a{i}", name=f"a{i}"))

            # prefetch every input (cast-DMA via SWDGE)
            for i in range(n_imgs):
                m3 = ms[i].rearrange("p (b c) -> p b c", c=FB)
                nc.gpsimd.dma_start(
                    m3[:, :, 2:FB - 2],
                    y_d[i].rearrange("(b p) w -> p b w", p=P))
                nc.gpsimd.dma_start(
                    xs[i].rearrange("p (b c) -> p b c", c=W),
                    x_d[i].rearrange("(b p) w -> p b w", p=P))

            for i in range(n_imgs):
                m, xb, ac = ms[i], xs[i], accs[i]
                m3 = m.rearrange("p (b c) -> p b c", c=FB)
                mc = m3[:, :, 2:FB - 2]          # center view [P, NB, W]
                x3 = xb.rearrange("p (b c) -> p b c", c=W)

                # outer-pair horizontal sum (bf16 2x; all views 4B-aligned)
                t = work.tile([P, FI], bf16, tag="t")
                nc.vector.tensor_add(t[:, 0:FI - 2], m[:, 0:FI - 2], m[:, 2:FI])
                t3 = t.rearrange("p (b c) -> p b c", c=FB)

                # zh = (m - 0.5) * x ;  l = softplus(-2*zh) = softplus((1-2m)x)
                zh = work.tile([P, FD], bf16, tag="zh")
                z3 = zh.rearrange("p (b c) -> p b c", c=W)
                nc.vector.scalar_tensor_tensor(z3, mc, 0.5, x3,
                                               Alu.subtract, Alu.mult)
                eb = work.tile([P, FD], bf16, tag="eb")
                nc.scalar.activation(eb[:], zh[:], Act.Exp, scale=-2.0)
                lt = work.tile([P, FD], bf16, tag="lt")
                nc.scalar.activation(lt[:], eb[:], Act.Ln, bias=1.0,
                                     accum_out=ac[:, 0:1])
                lt3 = lt.rearrange("p (b c) -> p b c", c=W)

                # vertical scaled 3-tap on PE, center tap folded in
                sp = pp.tile([P, FD], f32, tag="sp")

                def bank(b):
                    return sp[:, b * W:(b + 1) * W]

                def mm(b, wt, rhs, **kw):
                    nc.tensor.matmul(bank(b), wt, rhs, **kw)

                for b, wt in ((0, W_T0), (1, W_TRI), (2, W_TRI), (3, W_T3)):
                    mm(b, wt, t3[:, b, 1:FB - 3], start=True, stop=False)
                    mm(b, wt, mc[:, b], start=False, stop=False)
                for b in (0, 1, 2):
                    mm(b, W_U, t3[:, b + 1, 1:FB - 3], start=False, stop=False)
                    mm(b, W_U, mc[:, b + 1], start=False, stop=(b == 0))
                for b in (1, 2, 3):
                    mm(b, W_L, t3[:, b - 1, 1:FB - 3], start=False, stop=False)
                    mm(b, W_L, mc[:, b - 1], start=False, stop=(b != 0))

                # blocks 0..AB-1: ACT abs + one bf16 2x STT
                ab = work.tile([P, FA], bf16, tag="ab")
                nc.scalar.activation(ab[:], sp[:, 0:FA], Act.Abs, bias=bias_t[:])
                u1 = work.tile([P, FA], bf16, tag="u1")
                nc.vector.scalar_tensor_tensor(
                    u1[:], ab[:], 4.05, lt[:, 0:FA], Alu.is_le, Alu.mult,
                    accum_out=ac[:, 1:2])

                # remaining blocks: two one-sided STTs straight off PSUM
                u2 = work.tile([P, FD - FA], bf16, tag="u2")
                nc.vector.scalar_tensor_tensor(
                    u2[:], sp[:, FA:FD], 0.45, lt[:, FA:FD],
                    Alu.is_ge, Alu.mult, accum_out=ac[:, 2:3])
                u3 = work.tile([P, FD - FA], bf16, tag="u3")
                nc.vector.scalar_tensor_tensor(
                    u3[:], sp[:, FA:FD], 8.55, lt[:, FA:FD],
                    Alu.is_ge, Alu.mult, accum_out=ac[:, 3:4])

                # column 0/511 fix (handles corners too)
                sp3 = sp.rearrange("p (b c) -> p b c", c=W)
                ec = work.tile([P, 2 * NB], bf16, tag="ec")
                nc.vector.scalar_tensor_tensor(
                    ec.rearrange("p (b c) -> p b c", c=2),
                    sp3[:, :, ::W - 1], 5.5, lt3[:, :, ::W - 1],
                    Alu.is_ge, Alu.mult, accum_out=ac[:, 4:5])

                nc.sync.dma_start(acc_d[:, i * NACC:(i + 1) * NACC], ac[:])

    nc.compile()
    return nc


def _get_nc(n_imgs):
    if n_imgs not in _CACHE:
        _CACHE[n_imgs] = _build(n_imgs)
    return _CACHE[n_imgs]


def _combine(acc, n_imgs):
    a = acc.reshape(P, n_imgs, NACC).astype(np.float64)
    return (a[:, :, 0].sum() + a[:, :, 1].sum() + a[:, :, 2].sum()
            - a[:, :, 3].sum() - a[:, :, 4].sum())


def kernel(x, y):
    from concourse import bass_utils

    n = x.shape[0]
    per = n // N_CORES
    nc = _get_nc(per)
    wts = _consts()
    x = np.ascontiguousarray(x, dtype=np.float32)
    y = np.ascontiguousarray(y, dtype=np.int32)
    in_maps = [
        {"x": x[c * per:(c + 1) * per], "y": y[c * per:(c + 1) * per],
         "wts": wts}
        for c in range(N_CORES)
    ]
    res = bass_utils.run_bass_kernel_spmd(nc, in_maps, core_ids=list(range(N_CORES)))
    total = 0.0
    for r in res.results:
        total += _combine(r["acc"], per)
    return np.float32(total / (n * H * W))
